# revision 7
# baseline (speedup 1.0000x reference)
"""Trainium2 Bass kernel for AttentionLayerPooler.

Computes, for two independent weight/value streams (k and v):
    attn = softmax(logits)                  # [28, 36], tiny -> host
    pooled[m] = sum_l attn[m, l] * x[l]     # [28, B*H*S*D] matmul, device

Sharding: data-parallel over the H axis (16 heads -> 2 heads per core x 8
cores). Each core runs an identical program on its [36, 262144] f32 shard
of ks and vs, producing [28, 262144] outputs. No communication.

Device kernel strategy (memory-bound, ~134 MB traffic/core):
  - Contraction K=36 is tiny; pack 3 independent column-blocks of the free
    dim onto partitions (3*36=108) with a block-diagonal stationary weight
    lhsT [108, 84], so each matmul computes 3 output blocks at once and
    DMAs run with 108-partition tiles.
  - Weights stay stationary; rhs tiles stream [108, 512] -> PSUM [84, 512].
  - PSUM evicted to SBUF staging via DVE/ACT copies (alternating engines),
    then DMA'd out with the mirrored 3-block access pattern.
"""

import sys

sys.path.insert(0, "/opt/trn_rl_repo")

import numpy as np

import concourse.bass as bass
import concourse.tile as tile
from concourse import bacc, mybir
from concourse.bass_utils import run_bass_kernel_spmd

L = 36           # teacher layers (contraction dim)
M = 28           # student layers (output dim)
B, H, S, D = 1, 16, 1024, 128
N_CORES = 8
H_PER_CORE = H // N_CORES
NCOLS = H_PER_CORE * S * D          # 262144 columns per core per tensor
PACK = 3                            # column blocks packed on partitions
FW = 4096                           # tile free width (columns per block)
BODY = (NCOLS // (PACK * FW)) * PACK * FW   # 258048 = 21 tiles * 3 * 4096
NT = BODY // (PACK * FW)            # 21 body tiles per tensor
TAIL = NCOLS - BODY                 # 4096 remainder, processed unpacked
MMW = 512                           # matmul moving free dim (fp32 max)
HALF = 2048                         # psum tile width (4 banks)

FP32 = mybir.dt.float32
# float32r: identical 4-byte layout, but the PE runs matmuls at 1 cycle/row
# (vs 4 for float32) once the moving free dim is >=256. Reduced-precision
# multiply is far inside the 2e-2 tolerance.
FP32R = mybir.dt.float32r

# Engine-balanced "spread" layout: each 36-row input block is split into two
# 18-row chunks, each 28-row output block into two 14-row chunks, placed so
# DMA lines spread evenly over the 16 SDMA engines (max 13 lines/engine per
# tile vs 16 for the packed 0-107 layout; ideal is 12). Placement found by
# brute-force over the partition->engine swizzle.
IN_CHUNKS = [0, 21, 41, 65, 85, 106]     # 6 x 18 rows; block j = (j, j+3)
OUT_CHUNKS = [7, 27, 51, 70, 92, 114]    # 6 x 14 rows; block j = (j, j+3)
IN_PAIRS = [(IN_CHUNKS[j], IN_CHUNKS[j + 3]) for j in range(3)]
OUT_PAIRS = [(OUT_CHUNKS[j], OUT_CHUNKS[j + 3]) for j in range(3)]

_NC_CACHE = None


SPREAD = 0  # packed layout measured faster than the engine-spread variant

# Balanced contiguous-block layout: 3x36-row input blocks and 3x28-row
# output blocks at these partition offsets give a max per-SDMA-engine load
# of 14 lines/tile (vs 16 for the dense 0-107/0-83 packing; ideal 12),
# while keeping 6 dma_starts per tile. Gap rows carry zero weight
# rows/columns in the [128, 128] stationary operand.
BAL_IN = (0, 36, 74)
BAL_OUT = (0, 70, 98)


def _build_nc(reps=1, fw=4096, inbufs=8, stbufs=3, rot=0, psbufs=2, rott=1,
              spread=SPREAD, qmode=0, owide=1, rdt=1, balanced=0):
    global FW, BODY, NT, TAIL, HALF
    FW = fw
    HALF = min(2048, fw)
    BODY = (NCOLS // (PACK * FW)) * PACK * FW
    NT = BODY // (PACK * FW)
    TAIL = NCOLS - BODY
    nc = bacc.Bacc("TRN2", target_bir_lowering=False, debug=False,
                   num_devices=N_CORES)

    MDT = FP32R if rdt else FP32
    wshape = [128, 128] if (spread or balanced) else [PACK * L, PACK * M]
    k_in = nc.dram_tensor("k_in", [L, NCOLS], MDT, kind="ExternalInput")
    v_in = nc.dram_tensor("v_in", [L, NCOLS], MDT, kind="ExternalInput")
    w_k = nc.dram_tensor("w_k", wshape, MDT, kind="ExternalInput")
    w_v = nc.dram_tensor("w_v", wshape, MDT, kind="ExternalInput")
    if spread:
        w_k1 = nc.dram_tensor("w_k1", [L, M], FP32, kind="ExternalInput")
        w_v1 = nc.dram_tensor("w_v1", [L, M], FP32, kind="ExternalInput")
    k_out = nc.dram_tensor("k_out", [M, NCOLS], FP32, kind="ExternalOutput")
    v_out = nc.dram_tensor("v_out", [M, NCOLS], FP32, kind="ExternalOutput")

    if spread:
        return _build_spread_body(nc, reps, inbufs, stbufs, psbufs, rott,
                                  k_in, v_in, w_k, w_v, w_k1, w_v1,
                                  k_out, v_out, qmode)
    if balanced:
        return _build_balanced_body(nc, reps, inbufs, stbufs, psbufs, rot,
                                    rott, k_in, v_in, w_k, w_v, k_out, v_out,
                                    MDT, owide)

    with tile.TileContext(nc) as tc:
        with (
            tc.tile_pool(name="wpool", bufs=1) as wpool,
            tc.tile_pool(name="inpool", bufs=inbufs) as inpool,
            tc.tile_pool(name="stpool", bufs=stbufs) as stpool,
            tc.tile_pool(name="pspool", bufs=psbufs, space="PSUM") as pspool,
        ):
            wk = wpool.tile([PACK * L, PACK * M], MDT, tag="wk")
            nc.sync.dma_start(wk[:], w_k.ap()[:, :])
            wv = wpool.tile([PACK * L, PACK * M], MDT, tag="wv")
            nc.sync.dma_start(wv[:], w_v.ap()[:, :])

            if reps > 1:
                loop_cm = tc.For_i(0, reps, 1)
                loop_cm.__enter__()

            dma_engines = (nc.sync, nc.scalar, nc.gpsimd)
            # out-DMAs span `ow` consecutive input tiles (fewer, bigger
            # transfers; DMA instruction count is a measured cost here)
            if owide and NT % owide == 0:
                ow = owide
            elif owide and NT % 2 == 0:
                ow = 2
            else:
                ow = 1
            for x_in, x_out, w in ((k_in, k_out, wk), (v_in, v_out, wv)):
                xin_b = x_in.ap()[:, 0:BODY].rearrange("l (j c) -> j l c", j=PACK)
                xout_b = x_out.ap()[:, 0:BODY].rearrange("m (j c) -> j m c", j=PACK)
                for t in range(NT):
                    r = (t * rott) % PACK
                    tin = inpool.tile([PACK * L, FW], MDT)
                    for j in range(PACK):
                        dma_engines[(j + r) % PACK].dma_start(
                            tin[j * L:(j + 1) * L, :],
                            xin_b[j, :, t * FW:(t + 1) * FW])
                    if t % ow == 0:
                        stage = stpool.tile([PACK * M, ow * FW], FP32)
                    sh0 = (t % ow) * FW
                    for h in range(FW // HALF):
                        ps = pspool.tile([PACK * M, HALF], FP32)
                        for q in range(HALF // MMW):
                            c0 = h * HALF + q * MMW
                            nc.tensor.matmul(
                                ps[:, q * MMW:(q + 1) * MMW],
                                w[:, :],
                                tin[:, c0:c0 + MMW],
                                start=True, stop=True,
                            )
                        if h % 2 == 0:
                            nc.vector.tensor_copy(
                                stage[:, sh0 + h * HALF:sh0 + (h + 1) * HALF],
                                ps[:, :])
                        else:
                            nc.scalar.copy(
                                stage[:, sh0 + h * HALF:sh0 + (h + 1) * HALF],
                                ps[:, :])
                    if t % ow == ow - 1:
                        t0c = (t - ow + 1) * FW
                        for j in range(PACK):
                            dma_engines[(j + r + rot) % PACK].dma_start(
                                xout_b[j, :, t0c:t0c + ow * FW],
                                stage[j * M:(j + 1) * M, :])

                for tt in range(BODY, NCOLS, FW):
                    tw = min(FW, NCOLS - tt)
                    tin = inpool.tile([PACK * L, FW], MDT)
                    nc.sync.dma_start(tin[0:L, 0:tw], x_in.ap()[:, tt:tt + tw])
                    stage = stpool.tile([PACK * M, FW], FP32)
                    for h, hh in enumerate(range(0, tw, HALF)):
                        hw = min(HALF, tw - hh)
                        ps = pspool.tile([PACK * M, HALF], FP32)
                        for q in range(hw // MMW):
                            c0 = hh + q * MMW
                            nc.tensor.matmul(
                                ps[0:M, q * MMW:(q + 1) * MMW],
                                w[0:L, 0:M],
                                tin[0:L, c0:c0 + MMW],
                                start=True, stop=True,
                            )
                        if h % 2 == 0:
                            nc.vector.tensor_copy(
                                stage[0:M, hh:hh + hw], ps[0:M, 0:hw])
                        else:
                            nc.scalar.copy(
                                stage[0:M, hh:hh + hw], ps[0:M, 0:hw])
                    nc.sync.dma_start(x_out.ap()[:, tt:tt + tw], stage[0:M, 0:tw])

            if reps > 1:
                loop_cm.__exit__(None, None, None)

    nc.compile()
    return nc


def _build_spread_body(nc, reps, inbufs, stbufs, psbufs, rott,
                       k_in, v_in, w_k, w_v, w_k1, w_v1, k_out, v_out,
                       qmode=0):
    import concourse.tile as tile

    hl, hm = L // 2, M // 2
    # queue maps per chunk index: qmode 0 = even 3-way round-robin;
    # 1 = HWDGE only (sync/scalar); 2 = weighted, gpsimd gets 2 pieces
    if qmode == 0:
        in_q = [0, 1, 2, 0, 1, 2]
        out_q = [0, 1, 2, 0, 1, 2]
        nrot = 3
    elif qmode == 1:
        in_q = [0, 1, 0, 1, 0, 1]
        out_q = [1, 0, 1, 0, 1, 0]
        nrot = 2
    else:
        in_q = [0, 1, 0, 1, 0, 1]
        out_q = [2, 2, 0, 1, 0, 1]
        nrot = 2
    with tile.TileContext(nc) as tc:
        with (
            tc.tile_pool(name="wpool", bufs=1) as wpool,
            tc.tile_pool(name="stpool", bufs=stbufs) as stpool,
            tc.tile_pool(name="pspool", bufs=psbufs, space="PSUM") as pspool,
        ):
            wk = wpool.tile([128, 128], FP32, tag="wk")
            nc.sync.dma_start(wk[:], w_k.ap()[:, :])
            wv = wpool.tile([128, 128], FP32, tag="wv")
            nc.sync.dma_start(wv[:], w_v.ap()[:, :])
            wk1 = wpool.tile([L, M], FP32, tag="wk1")
            nc.sync.dma_start(wk1[:], w_k1.ap()[:, :])
            wv1 = wpool.tile([L, M], FP32, tag="wv1")
            nc.sync.dma_start(wv1[:], w_v1.ap()[:, :])

            # manual input ring: raw SBUF tensors, memset once (gap rows
            # must be finite: they hit zero weight rows in the matmul)
            tinb = [nc.alloc_sbuf_tensor(f"tinb{i}", [128, FW], FP32)
                    for i in range(inbufs)]
            for tb in tinb:
                nc.gpsimd.memset(tb.ap()[:, :], 0.0)

            dma_engines = (nc.sync, nc.scalar, nc.gpsimd)
            if reps > 1:
                loop_cm = tc.For_i(0, reps, 1)
                loop_cm.__enter__()

            tctr = 0
            for x_in, x_out, w, w1 in ((k_in, k_out, wk, wk1),
                                       (v_in, v_out, wv, wv1)):
                for t in range(NT):
                    r = (t * rott) % nrot
                    tb = tinb[tctr % inbufs].ap()
                    tctr += 1
                    for c in range(2 * PACK):
                        j, half = c % PACK, c // PACK
                        s = IN_CHUNKS[c]
                        col = j * (BODY // PACK) + t * FW
                        q = in_q[c]
                        q = (q + r) % 3 if nrot == 3 else (
                            (q + r) % 2 if q < 2 else q)
                        dma_engines[q].dma_start(
                            tb[s:s + hl, :],
                            x_in.ap()[half * hl:(half + 1) * hl,
                                      col:col + FW])
                    stage = stpool.tile([128, FW], FP32)
                    for h in range(FW // HALF):
                        ps = pspool.tile([128, HALF], FP32)
                        for q in range(HALF // MMW):
                            c0 = h * HALF + q * MMW
                            nc.tensor.matmul(
                                ps[:, q * MMW:(q + 1) * MMW],
                                w[:, :], tb[:, c0:c0 + MMW],
                                start=True, stop=True)
                        if h % 2 == 0:
                            nc.vector.tensor_copy(
                                stage[:, h * HALF:(h + 1) * HALF], ps[:, :])
                        else:
                            nc.scalar.copy(
                                stage[:, h * HALF:(h + 1) * HALF], ps[:, :])
                    for c in range(2 * PACK):
                        j, half = c % PACK, c // PACK
                        o = OUT_CHUNKS[c]
                        col = j * (BODY // PACK) + t * FW
                        q = out_q[c]
                        q = (q + r) % 3 if nrot == 3 else (
                            (q + r) % 2 if q < 2 else q)
                        dma_engines[q].dma_start(
                            x_out.ap()[half * hm:(half + 1) * hm,
                                       col:col + FW],
                            stage[o:o + hm, :])

                for tt in range(BODY, NCOLS, FW):
                    tw = min(FW, NCOLS - tt)
                    tb = tinb[tctr % inbufs].ap()
                    tctr += 1
                    nc.sync.dma_start(tb[0:L, 0:tw], x_in.ap()[:, tt:tt + tw])
                    stage = stpool.tile([128, FW], FP32)
                    for h, hh in enumerate(range(0, tw, HALF)):
                        hw = min(HALF, tw - hh)
                        ps = pspool.tile([128, HALF], FP32)
                        for q in range(hw // MMW):
                            c0 = hh + q * MMW
                            nc.tensor.matmul(
                                ps[0:M, q * MMW:(q + 1) * MMW],
                                w1[:, :], tb[0:L, c0:c0 + MMW],
                                start=True, stop=True)
                        if h % 2 == 0:
                            nc.vector.tensor_copy(
                                stage[0:M, hh:hh + hw], ps[0:M, 0:hw])
                        else:
                            nc.scalar.copy(
                                stage[0:M, hh:hh + hw], ps[0:M, 0:hw])
                    nc.sync.dma_start(x_out.ap()[:, tt:tt + tw],
                                      stage[0:M, 0:tw])

            if reps > 1:
                loop_cm.__exit__(None, None, None)

    nc.compile()
    return nc


def _get_nc():
    global _NC_CACHE
    if _NC_CACHE is None:
        _NC_CACHE = _build_nc()
    return _NC_CACHE


def _softmax_f32(x):
    x = np.asarray(x, np.float32)
    x = x - x.max(axis=-1, keepdims=True)
    e = np.exp(x)
    return (e / e.sum(axis=-1, keepdims=True)).astype(np.float32)


def _block_diag_lhsT(attn):
    # lhsT[36j + l, 28j + m] = attn[m, l]; out = lhsT.T @ rhs
    w = np.zeros((PACK * L, PACK * M), np.float32)
    wt = np.ascontiguousarray(attn.T)  # [36, 28]
    for j in range(PACK):
        w[j * L:(j + 1) * L, j * M:(j + 1) * M] = wt
    return w


def _spread_lhsT(attn):
    # lhsT[row(l,j), col(m,j)] = attn[m, l] with rows/cols at the
    # engine-balanced chunk positions; zeros elsewhere
    w = np.zeros((128, 128), np.float32)
    at = np.ascontiguousarray(attn.T)  # [36, 28]
    hl, hm = L // 2, M // 2
    for j in range(PACK):
        s1, s2 = IN_PAIRS[j]
        o1, o2 = OUT_PAIRS[j]
        w[s1:s1 + hl, o1:o1 + hm] = at[0:hl, 0:hm]
        w[s1:s1 + hl, o2:o2 + hm] = at[0:hl, hm:M]
        w[s2:s2 + hl, o1:o1 + hm] = at[hl:L, 0:hm]
        w[s2:s2 + hl, o2:o2 + hm] = at[hl:L, hm:M]
    return w


def kernel(ks, vs, attn_logits_k, attn_logits_v, _trace=False, _trace_kwargs=None):
    ks = np.asarray(ks, np.float32)
    vs = np.asarray(vs, np.float32)
    ak = _softmax_f32(attn_logits_k)
    av = _softmax_f32(attn_logits_v)

    nc = _get_nc()

    if SPREAD:
        wextra = {"w_k": _spread_lhsT(ak), "w_v": _spread_lhsT(av),
                  "w_k1": np.ascontiguousarray(ak.T),
                  "w_v1": np.ascontiguousarray(av.T)}
    else:
        wextra = {"w_k": _block_diag_lhsT(ak), "w_v": _block_diag_lhsT(av)}
    in_maps = []
    for c in range(N_CORES):
        h0 = c * H_PER_CORE
        in_maps.append({
            "k_in": np.ascontiguousarray(
                ks[:, 0, h0:h0 + H_PER_CORE]).reshape(L, NCOLS),
            "v_in": np.ascontiguousarray(
                vs[:, 0, h0:h0 + H_PER_CORE]).reshape(L, NCOLS),
            **wextra,
        })

    res = run_bass_kernel_spmd(
        nc, in_maps, core_ids=list(range(N_CORES)),
        trace=_trace, **(_trace_kwargs or {}),
    )

    ks_pooled = np.empty((M, B, H, S, D), np.float32)
    vs_pooled = np.empty((M, B, H, S, D), np.float32)
    for c in range(N_CORES):
        h0 = c * H_PER_CORE
        ks_pooled[:, 0, h0:h0 + H_PER_CORE] = (
            res.results[c]["k_out"].reshape(M, H_PER_CORE, S, D))
        vs_pooled[:, 0, h0:h0 + H_PER_CORE] = (
            res.results[c]["v_out"].reshape(M, H_PER_CORE, S, D))

    if _trace:
        return (ks_pooled, vs_pooled), res
    return (ks_pooled, vs_pooled)

